# revision 13
# baseline (speedup 1.0000x reference)
"""HGCN layer kernel for Trainium2, 8 NeuronCores, row-sharded SPMD.

Reference computation (N=6144, D=512):
    type_sum_a = adj_a @ x ; type_sum_b = adj_b @ x
    attn_a = sigmoid(cat[ts_a, x] @ Wa.T + ba) ; attn_b likewise
    h = x @ W_sa ; s_l = h @ a_sa[:512] ; s_r = h @ a_sa[512:]
    scores[i,j] = s_l[i] + s_r[j]
    e = adj_a * exp(-leaky_relu(scores, 0.01)) ; attn = e / (rowsum(e)+1e-5)
    x_a = attn @ h ; x_b = adj_b @ (x @ W_gcnb) + b_gcnb
    out = sigmoid(attn_a * x_a + attn_b * x_b)

Kernel strategy (per core, NL=768 local rows):
  - R = [W_sa | W_gcnb | W_sa@a_l | W_sa@a_r | Wa1.T | Wb1.T | Wa2.T | Wb2.T]
    HX = x @ R computed replicated on every core (needs full h/xW anyway).
    Gates reassociate (adj@x)@W1.T -> adj@(x@W1.T) so the NxN gate matmuls
    shrink to N-vector contractions folded into PE side-passes.
  - e computed in transposed layout [j(part), i(free)] so it is directly the
    lhsT of the attention matmul; adjacency is passed in pre-transposed,
    per-core-permuted (local rows first) so one SPMD program serves all cores.
  - rowsum(e) via ones-vector lhsT pass; division applied after the matmul.
  - float32r matmuls (4x fp32 PE rate at N>=256).
"""

import numpy as np
from contextlib import ExitStack

import concourse.bass as bass
import concourse.bacc as bacc
import concourse.mybir as mybir
import concourse.tile as tile

F32 = mybir.dt.float32
F32R = mybir.dt.float32r
AF = mybir.ActivationFunctionType
ALU = mybir.AluOpType

N_CORES = 8


def _chunks(total, size=512):
    out = []
    o = 0
    while o < total:
        out.append((o, min(size, total - o)))
        o += size
    return out


def build_program(n, d, nl, ba, bb, mm_dt=F32R, lrelu_on_act=False):
    """Build the SPMD Bass program. Returns nc.

    n: total nodes, d: feature dim, nl: local rows per core.
    ba/bb: python-float gate biases (baked in).
    """
    JT = n // 128   # j tiles (contraction/node axis)
    IT = nl // 128  # local row tiles
    KT = d // 128   # feature k tiles
    NR = 2 * d + 8  # columns of R
    # stats cols: 0=s_l 1=s_r 2=va 3=vb 4=wa2x 5=wb2x 6,7=pad

    nc = bacc.Bacc("TRN2", target_bir_lowering=False, debug=False,
                   num_devices=N_CORES)

    xt_dram = nc.dram_tensor("xt", [JT, KT, 128, 128], mm_dt, kind="ExternalInput")
    r_dram = nc.dram_tensor("rmat", [KT, 128, NR], mm_dt, kind="ExternalInput")
    adjat_dram = nc.dram_tensor("adjat", [JT, 128, nl], mm_dt, kind="ExternalInput")
    adjbt_dram = nc.dram_tensor("adjbt", [JT, 128, nl], mm_dt, kind="ExternalInput")
    bbias_dram = nc.dram_tensor("bbias", [128, d], F32, kind="ExternalInput")
    ident_dram = nc.dram_tensor("ident", [128, 128], F32, kind="ExternalInput")
    out_dram = nc.dram_tensor("out", [nl, d], F32, kind="ExternalOutput")

    xw_dram = nc.dram_tensor("xw_scratch", [JT, 128, d], mm_dt)

    def mm(out, lhsT, rhs, start, stop, skip_group_check=False):
        nc.tensor.matmul(out, lhsT, rhs, start=start, stop=stop,
                         skip_group_check=skip_group_check)

    with tile.TileContext(nc) as tc, ExitStack() as ctx:
        const = ctx.enter_context(tc.tile_pool(name="const", bufs=1))

        r_sb = const.tile([128, KT, NR], mm_dt, tag="r")
        h_sb = const.tile([128, JT * d], mm_dt, tag="h")
        stats_sb = const.tile([128, JT * 8], F32, tag="stats")
        stats_r = const.tile([128, JT * 8], mm_dt, tag="stats_r")
        slb_sb = const.tile([128, nl], F32, tag="slb")
        xb_sb = const.tile([128, IT * d], F32, tag="xb")
        xa_sb = const.tile([128, IT * d], F32, tag="xa")
        bbias_sb = const.tile([128, d], F32, tag="bbias")
        ident_sb = const.tile([128, 128], F32, tag="ident")
        ones2 = const.tile([128, 2], mm_dt, tag="ones2")
        ones2_f = const.tile([128, 2], F32, tag="ones2f")
        ones_row = const.tile([1, 128], F32, tag="ones_r")
        neg1 = const.tile([128, 1], F32, tag="neg1")
        ba_sb = const.tile([128, 1], F32, tag="ba")
        bb_sb = const.tile([128, 1], F32, tag="bb")
        sl_row = const.tile([1, nl], F32, tag="sl_row")
        g_rs = const.tile([128, 2 * IT], F32, tag="g_rs")  # rs at col 2i
        g_ga = const.tile([128, 2 * IT], F32, tag="g_ga")  # ga at col 2i
        g_gb = const.tile([128, 2 * IT], F32, tag="g_gb")  # gb at col 2i+1
        gate_sb = const.tile([128, 4 * IT], F32, tag="gate")
        # gate_sb cols: [0:IT]=recip(rowsum), [IT:2IT]=sig_a, [2IT:3IT]=sig_b,
        # [3IT:4IT]=scratch

        for k in range(KT):
            nc.sync.dma_start(out=r_sb[:, k, :], in_=r_dram[k])
        nc.sync.dma_start(out=bbias_sb[:], in_=bbias_dram[:])
        nc.sync.dma_start(out=ident_sb[:], in_=ident_dram[:])
        nc.vector.memset(ones2_f[:], 1.0)
        nc.vector.tensor_copy(ones2[:], ones2_f[:])
        nc.vector.memset(ones_row[:], 1.0)
        nc.vector.memset(neg1[:], -1.0)
        nc.vector.memset(ba_sb[:], float(ba))
        nc.vector.memset(bb_sb[:], float(bb))

        # ---- Phase A: HX = x @ R (replicated over all n nodes) ----
        with tc.tile_pool(name="xt_pool", bufs=3) as xtp, \
             tc.tile_pool(name="xw_out", bufs=3) as xwop, \
             tc.tile_pool(name="psA", bufs=2, space="PSUM") as psA:
            for m in range(JT):
                xt_t = xtp.tile([128, KT * 128], mm_dt, tag="xt")
                for k in range(KT):
                    nc.sync.dma_start(out=xt_t[:, k * 128:(k + 1) * 128],
                                      in_=xt_dram[m, k])
                ph = psA.tile([128, d], F32, tag="ph")
                pw = psA.tile([128, d], F32, tag="pw")
                ps = psA.tile([128, 8], F32, tag="ps")
                for k in range(KT):
                    lhsT = xt_t[:, k * 128:(k + 1) * 128]
                    st, sp = (k == 0), (k == KT - 1)
                    mm(ph[:], lhsT, r_sb[:, k, 0:d], st, sp)
                    mm(pw[:], lhsT, r_sb[:, k, d:2 * d], st, sp)
                    mm(ps[:], lhsT, r_sb[:, k, 2 * d:NR], st, sp)
                nc.scalar.copy(h_sb[:, m * d:(m + 1) * d], ph[:])
                xw_t = xwop.tile([128, d], mm_dt, tag="xwo")
                nc.scalar.copy(xw_t[:], pw[:])
                nc.sync.dma_start(out=xw_dram[m], in_=xw_t[:])
                nc.vector.tensor_copy(stats_sb[:, m * 8:(m + 1) * 8], ps[:])
                nc.vector.tensor_copy(stats_r[:, m * 8:(m + 1) * 8], ps[:])

        # ---- Phase A2: build SL broadcast [128, nl] from local s_l ----
        with tc.tile_pool(name="psA2", bufs=1, space="PSUM") as psA2:
            ch = _chunks(nl)
            ptrs = [psA2.tile([1, c[1]], F32, tag=f"psl{ci}",
                              name=f"psl{ci}")
                    for ci, c in enumerate(ch)]
            for t in range(IT):
                ci, off = divmod(t * 128, 512)
                # transpose stats col (s_l of local tile t) -> row chunk
                nc.tensor.matmul(ptrs[ci][0:1, off:off + 128],
                                 stats_sb[:, t * 8:t * 8 + 1],
                                 ident_sb[:], start=True, stop=True)
            for ci, (o, w) in enumerate(ch):
                nc.vector.tensor_copy(sl_row[0:1, o:o + w], ptrs[ci][0:1, :])
            for ci, (o, w) in enumerate(ch):
                pb = psA2.tile([128, w], F32, tag="pslb")
                nc.tensor.matmul(pb[:], ones_row[:], sl_row[0:1, o:o + w],
                                 start=True, stop=True)
                nc.vector.tensor_copy(slb_sb[:, o:o + w], pb[:])

        # ---- Phase B: x_b = adj_b @ xW ; gb = vb^T adj_bT ----
        with tc.tile_pool(name="adjB", bufs=3) as adjp, \
             tc.tile_pool(name="rhsB", bufs=3) as rhsp, \
             tc.tile_pool(name="psB", bufs=1, space="PSUM") as psB:
            pb_acc = [psB.tile([128, d], F32, tag=f"pb{i}", name=f"pb{i}")
                      for i in range(IT)]
            # gb[i] = sum_j adj_b[i,j] vb[j]: column per i-block, one bank;
            # single start/stop for the whole bank (per-bank zeroing).
            pgb = psB.tile([128, 2 * IT], F32, tag="pgb", name="pgb")
            for j in range(JT):
                at = adjp.tile([128, nl], mm_dt, tag="adj")
                nc.sync.dma_start(out=at[:], in_=adjbt_dram[j])
                xw_t = rhsp.tile([128, d], mm_dt, tag="xw")
                nc.sync.dma_start(out=xw_t[:], in_=xw_dram[j])
                st, sp = (j == 0), (j == JT - 1)
                vab = stats_r[:, j * 8 + 2:j * 8 + 4]
                for i in range(IT):
                    mm(pb_acc[i][:], at[:, i * 128:(i + 1) * 128], xw_t[:], st, sp)
                    mm(pgb[:, 2 * i:2 * i + 2], at[:, i * 128:(i + 1) * 128],
                       vab, st and i == 0, sp and i == IT - 1)
            for i in range(IT):
                nc.scalar.copy(xb_sb[:, i * d:(i + 1) * d], pb_acc[i][:])
            nc.vector.tensor_copy(g_gb[:], pgb[:])

        # ---- Phase C: e = adj_a * exp(-lrelu(s)); y_a = e^T.T @ h;
        #      rowsum + ga side-passes ----
        with tc.tile_pool(name="adjC", bufs=3) as adjp, \
             tc.tile_pool(name="ewC", bufs=3) as ewp, \
             tc.tile_pool(name="psC", bufs=1, space="PSUM") as psC:
            pc_acc = [psC.tile([128, d], F32, tag=f"pc{i}", name=f"pc{i}")
                      for i in range(IT)]
            # rowsum/ga as columns per i-block (lhsT shared with pc_acc)
            prs = psC.tile([128, 2 * IT], F32, tag="prs", name="prs")
            pga = psC.tile([128, 2 * IT], F32, tag="pga", name="pga")
            for j in range(JT):
                at = adjp.tile([128, nl], mm_dt, tag="adj")
                nc.sync.dma_start(out=at[:], in_=adjat_dram[j])
                s_r = stats_sb[:, j * 8 + 1:j * 8 + 2]
                m_t = ewp.tile([128, nl], F32, tag="m")
                if lrelu_on_act:
                    nc.scalar.activation(m_t[:], slb_sb[:], AF.Prelu,
                                         bias=s_r, alpha=0.01)
                else:
                    nc.vector.tensor_scalar_add(m_t[:], slb_sb[:], s_r)
                    nc.vector.scalar_tensor_tensor(m_t[:], m_t[:], 0.01, m_t[:],
                                                   op0=ALU.mult, op1=ALU.max)
                # w = exp(-m), in place
                nc.scalar.activation(m_t[:], m_t[:], AF.Exp, scale=neg1[:])
                e_t = ewp.tile([128, nl], mm_dt, tag="e")
                nc.vector.tensor_tensor(e_t[:], m_t[:], at[:], op=ALU.mult)
                st, sp = (j == 0), (j == JT - 1)
                vab = stats_r[:, j * 8 + 2:j * 8 + 4]
                for i in range(IT):
                    mm(pc_acc[i][:], e_t[:, i * 128:(i + 1) * 128],
                       h_sb[:, j * d:(j + 1) * d], st, sp)
                    mm(prs[:, 2 * i:2 * i + 2], e_t[:, i * 128:(i + 1) * 128],
                       ones2[:], st and i == 0, sp and i == IT - 1)
                    mm(pga[:, 2 * i:2 * i + 2], at[:, i * 128:(i + 1) * 128],
                       vab, st and i == 0, sp and i == IT - 1)
            for i in range(IT):
                nc.scalar.copy(xa_sb[:, i * d:(i + 1) * d], pc_acc[i][:])
            nc.vector.tensor_copy(g_rs[:], prs[:])
            nc.vector.tensor_copy(g_ga[:], pga[:])

        # ---- Phase D: gates + combine (g_sb cols: rs | ga | gb) ----
        with tc.tile_pool(name="outD", bufs=2) as outp:
            for i in range(IT):
                # recip(rowsum + 1e-5)
                nc.vector.tensor_scalar_add(gate_sb[:, 3 * IT + i:3 * IT + i + 1],
                                            g_rs[:, 2 * i:2 * i + 1], 1e-5)
                nc.vector.reciprocal(gate_sb[:, i:i + 1],
                                     gate_sb[:, 3 * IT + i:3 * IT + i + 1])
                # sig_a = sigmoid(ga + wa2x + ba)
                nc.vector.tensor_tensor(gate_sb[:, 3 * IT + i:3 * IT + i + 1],
                                        g_ga[:, 2 * i:2 * i + 1],
                                        stats_sb[:, i * 8 + 4:i * 8 + 5],
                                        op=ALU.add)
                nc.scalar.activation(gate_sb[:, IT + i:IT + i + 1],
                                     gate_sb[:, 3 * IT + i:3 * IT + i + 1],
                                     AF.Sigmoid, bias=ba_sb[:])
                # sig_b = sigmoid(gb + wb2x + bb)
                nc.vector.tensor_tensor(gate_sb[:, 3 * IT + i:3 * IT + i + 1],
                                        g_gb[:, 2 * i + 1:2 * i + 2],
                                        stats_sb[:, i * 8 + 5:i * 8 + 6],
                                        op=ALU.add)
                nc.scalar.activation(gate_sb[:, 2 * IT + i:2 * IT + i + 1],
                                     gate_sb[:, 3 * IT + i:3 * IT + i + 1],
                                     AF.Sigmoid, bias=bb_sb[:])
            for i in range(IT):
                u_t = outp.tile([128, d], F32, tag="u")
                # u = sig_a * (x_a_raw * recip)
                nc.vector.tensor_scalar(u_t[:], xa_sb[:, i * d:(i + 1) * d],
                                        gate_sb[:, i:i + 1],
                                        gate_sb[:, IT + i:IT + i + 1],
                                        op0=ALU.mult, op1=ALU.mult)
                t_t = outp.tile([128, d], F32, tag="t")
                # t = x_b_raw + b_gcnb
                nc.vector.tensor_tensor(t_t[:], xb_sb[:, i * d:(i + 1) * d],
                                        bbias_sb[:], op=ALU.add)
                # y = sigmoid(t * sig_b + u)
                nc.vector.scalar_tensor_tensor(t_t[:], t_t[:],
                                               gate_sb[:, 2 * IT + i:2 * IT + i + 1],
                                               u_t[:], op0=ALU.mult, op1=ALU.add)
                y_t = outp.tile([128, d], F32, tag="y")
                nc.scalar.activation(y_t[:], t_t[:], AF.Sigmoid)
                nc.sync.dma_start(out=out_dram[i * 128:(i + 1) * 128, :],
                                  in_=y_t[:])

    nc.compile()
    return nc


def make_r_matrix(W_sa, a_sa, W_gcnb, Wa, Wb, d):
    cols = np.zeros((d, 8), dtype=np.float32)
    cols[:, 0] = W_sa @ a_sa[0, :d]
    cols[:, 1] = W_sa @ a_sa[0, d:]
    cols[:, 2] = Wa[0, :d]
    cols[:, 3] = Wb[0, :d]
    cols[:, 4] = Wa[0, d:]
    cols[:, 5] = Wb[0, d:]
    return np.ascontiguousarray(
        np.concatenate([W_sa, W_gcnb, cols], axis=1)).astype(np.float32)


def make_core_inputs(x, adj_a, adj_b, R, b_gcnb, n, d, nl, core):
    JT, KT = n // 128, d // 128
    rows = np.arange(core * nl, (core + 1) * nl)
    perm = np.concatenate([rows, np.arange(0, core * nl),
                           np.arange((core + 1) * nl, n)])
    xp = x[perm]
    xt = np.ascontiguousarray(
        xp.reshape(JT, 128, KT, 128).transpose(0, 2, 3, 1))
    adjat = np.ascontiguousarray(adj_a[rows][:, perm].T).reshape(JT, 128, nl)
    adjbt = np.ascontiguousarray(adj_b[rows][:, perm].T).reshape(JT, 128, nl)
    return {
        "xt": xt.astype(np.float32),
        "rmat": R.reshape(KT, 128, 2 * d + 8),
        "adjat": adjat.astype(np.float32),
        "adjbt": adjbt.astype(np.float32),
        "bbias": np.ascontiguousarray(
            np.broadcast_to(b_gcnb, (128, d))).astype(np.float32),
        "ident": np.eye(128, dtype=np.float32),
    }


_CACHE = {}


def _install_ntff_hook():
    """Dev-only: register the axon NTFF profile hook so trace=True works.

    The agent image's antenv package lacks axon_hooks; synthesize it and
    wire trn_boot's ctypes-based hook to /opt/axon/libaxon_pjrt.so.
    """
    import sys
    import types
    try:
        from antenv import axon_hooks  # noqa: F401
        return
    except ImportError:
        pass
    import antenv
    mod = types.ModuleType("antenv.axon_hooks")
    _h = [None]
    mod.get_axon_ntff_profile_hook = lambda: _h[0]
    mod.set_axon_ntff_profile_hook = lambda hook: _h.__setitem__(0, hook)
    sys.modules["antenv.axon_hooks"] = mod
    antenv.axon_hooks = mod
    from trn_agent_boot.trn_boot import _ntff_profile_via_ctypes
    mod.set_axon_ntff_profile_hook(
        _ntff_profile_via_ctypes("/opt/axon/libaxon_pjrt.so"))


def kernel(x, adj_a, adj_b, W_sa, a_sa, W_gcnb, b_gcnb, Wa, ba, Wb, bb,
           _trace=False, _trace_kwargs=None):
    from concourse.bass_utils import run_bass_kernel_spmd
    if _trace:
        _install_ntff_hook()

    n, d = x.shape
    nl = n // N_CORES
    R = make_r_matrix(W_sa, a_sa, W_gcnb, Wa, Wb, d)

    key = (n, d, nl, float(ba[0]), float(bb[0]))
    if key not in _CACHE:
        _CACHE[key] = build_program(n, d, nl, float(ba[0]), float(bb[0]))
    nc = _CACHE[key]

    in_maps = [make_core_inputs(x, adj_a, adj_b, R, b_gcnb, n, d, nl, c)
               for c in range(N_CORES)]
    res = run_bass_kernel_spmd(nc, in_maps, list(range(N_CORES)),
                               trace=_trace, **(_trace_kwargs or {}))
    out = np.empty((n, d), dtype=np.float32)
    for c in range(N_CORES):
        out[c * nl:(c + 1) * nl] = res.results[c]["out"]
    if _trace:
        kernel._last_results = res
    return out


# revision 15
# speedup vs baseline: 1.3148x; 1.3148x over previous
"""HGCN layer kernel for Trainium2, 8 NeuronCores, row-sharded SPMD.

Reference computation (N=6144, D=512):
    type_sum_a = adj_a @ x ; type_sum_b = adj_b @ x
    attn_a = sigmoid(cat[ts_a, x] @ Wa.T + ba) ; attn_b likewise
    h = x @ W_sa ; s_l = h @ a_sa[:512] ; s_r = h @ a_sa[512:]
    scores[i,j] = s_l[i] + s_r[j]
    e = adj_a * exp(-leaky_relu(scores, 0.01)) ; attn = e / (rowsum(e)+1e-5)
    x_a = attn @ h ; x_b = adj_b @ (x @ W_gcnb) + b_gcnb
    out = sigmoid(attn_a * x_a + attn_b * x_b)

Kernel strategy (per core, NL=768 local rows):
  - R = [W_sa | W_gcnb | W_sa@a_l | W_sa@a_r | Wa1.T | Wb1.T | Wa2.T | Wb2.T]
    HX = x @ R computed replicated on every core (needs full h/xW anyway).
    Gates reassociate (adj@x)@W1.T -> adj@(x@W1.T) so the NxN gate matmuls
    shrink to N-vector contractions folded into PE side-passes.
  - e computed in transposed layout [j(part), i(free)] so it is directly the
    lhsT of the attention matmul; adjacency is passed in pre-transposed,
    per-core-permuted (local rows first) so one SPMD program serves all cores.
  - rowsum(e) via ones-vector lhsT pass; division applied after the matmul.
  - float32r matmuls (4x fp32 PE rate at N>=256).
"""

import numpy as np
from contextlib import ExitStack

import concourse.bass as bass
import concourse.bacc as bacc
import concourse.mybir as mybir
import concourse.tile as tile

F32 = mybir.dt.float32
F32R = mybir.dt.float32r
AF = mybir.ActivationFunctionType
ALU = mybir.AluOpType

N_CORES = 8


def _chunks(total, size=512):
    out = []
    o = 0
    while o < total:
        out.append((o, min(size, total - o)))
        o += size
    return out


def build_program(n, d, nl, ba, bb, mm_dt=F32R, lrelu_on_act=False):
    """Build the SPMD Bass program. Returns nc.

    n: total nodes, d: feature dim, nl: local rows per core.
    ba/bb: python-float gate biases (baked in).
    """
    JT = n // 128   # j tiles (contraction/node axis)
    IT = nl // 128  # local row tiles
    KT = d // 128   # feature k tiles
    NR = 2 * d + 8  # columns of R
    # stats cols: 0=s_l 1=s_r 2=zero 3=va 4=vb 5=wa2x 6=wb2x 7=pad

    nc = bacc.Bacc("TRN2", target_bir_lowering=False, debug=False,
                   num_devices=N_CORES)

    xt_dram = nc.dram_tensor("xt", [JT, KT, 128, 128], mm_dt, kind="ExternalInput")
    r_dram = nc.dram_tensor("rmat", [KT, 128, NR], mm_dt, kind="ExternalInput")
    adjat_dram = nc.dram_tensor("adjat", [JT, 128, nl], mm_dt, kind="ExternalInput")
    adjbt_dram = nc.dram_tensor("adjbt", [JT, 128, nl], mm_dt, kind="ExternalInput")
    bbias_dram = nc.dram_tensor("bbias", [128, d], F32, kind="ExternalInput")
    ident_dram = nc.dram_tensor("ident", [128, 128], F32, kind="ExternalInput")
    out_dram = nc.dram_tensor("out", [nl, d], F32, kind="ExternalOutput")

    xw_dram = nc.dram_tensor("xw_scratch", [JT, 128, d], mm_dt)

    def mm(out, lhsT, rhs, start, stop, skip_group_check=False):
        nc.tensor.matmul(out, lhsT, rhs, start=start, stop=stop,
                         skip_group_check=skip_group_check)

    with tile.TileContext(nc) as tc, ExitStack() as ctx:
        const = ctx.enter_context(tc.tile_pool(name="const", bufs=1))

        r_sb = const.tile([128, KT, NR], mm_dt, tag="r")
        h_sb = const.tile([128, JT * d], mm_dt, tag="h")
        stats_sb = const.tile([128, JT * 8], F32, tag="stats")
        stats_r = const.tile([128, JT * 8], mm_dt, tag="stats_r")
        slb_sb = const.tile([128, nl], F32, tag="slb")
        xb_sb = const.tile([128, IT * d], F32, tag="xb")
        xa_sb = const.tile([128, IT * d], F32, tag="xa")
        bbias_sb = const.tile([128, d], F32, tag="bbias")
        ident_sb = const.tile([128, 128], F32, tag="ident")
        onespad = const.tile([128, 2], mm_dt, tag="onespad")
        onespad_f = const.tile([128, 2], F32, tag="onespadf")
        ones_row = const.tile([1, 128], F32, tag="ones_r")
        neg1 = const.tile([128, 1], F32, tag="neg1")
        ba_sb = const.tile([128, 1], F32, tag="ba")
        bb_sb = const.tile([128, 1], F32, tag="bb")
        sl_row = const.tile([1, nl], F32, tag="sl_row")
        g_sb = const.tile([128, 3 * IT], F32, tag="g")  # rs|ga|gb cols
        rg_rows = const.tile([2, nl], F32, tag="rg_rows")  # row0=rs row1=ga
        gb_row = const.tile([1, nl], F32, tag="gb_row")
        gate_sb = const.tile([128, 4 * IT], F32, tag="gate")
        # gate_sb cols: [0:IT]=recip(rowsum), [IT:2IT]=sig_a, [2IT:3IT]=sig_b,
        # [3IT:4IT]=scratch

        for k in range(KT):
            nc.sync.dma_start(out=r_sb[:, k, :], in_=r_dram[k])
        nc.sync.dma_start(out=bbias_sb[:], in_=bbias_dram[:])
        nc.sync.dma_start(out=ident_sb[:], in_=ident_dram[:])
        nc.vector.memset(onespad_f[:], 0.0)
        nc.vector.memset(onespad_f[:, 0:1], 1.0)
        nc.vector.tensor_copy(onespad[:], onespad_f[:])
        nc.vector.memset(ones_row[:], 1.0)
        nc.vector.memset(neg1[:], -1.0)
        nc.vector.memset(ba_sb[:], float(ba))
        nc.vector.memset(bb_sb[:], float(bb))

        # ---- Phase A: HX = x @ R (replicated over all n nodes) ----
        with tc.tile_pool(name="xt_pool", bufs=3) as xtp, \
             tc.tile_pool(name="xw_out", bufs=3) as xwop, \
             tc.tile_pool(name="psA", bufs=2, space="PSUM") as psA:
            for m in range(JT):
                xt_t = xtp.tile([128, KT * 128], mm_dt, tag="xt")
                for k in range(KT):
                    nc.sync.dma_start(out=xt_t[:, k * 128:(k + 1) * 128],
                                      in_=xt_dram[m, k])
                ph = psA.tile([128, d], F32, tag="ph")
                pw = psA.tile([128, d], F32, tag="pw")
                ps = psA.tile([128, 8], F32, tag="ps")
                for k in range(KT):
                    lhsT = xt_t[:, k * 128:(k + 1) * 128]
                    st, sp = (k == 0), (k == KT - 1)
                    mm(ph[:], lhsT, r_sb[:, k, 0:d], st, sp)
                    mm(pw[:], lhsT, r_sb[:, k, d:2 * d], st, sp)
                    mm(ps[:], lhsT, r_sb[:, k, 2 * d:NR], st, sp)
                nc.scalar.copy(h_sb[:, m * d:(m + 1) * d], ph[:])
                xw_t = xwop.tile([128, d], mm_dt, tag="xwo")
                nc.scalar.copy(xw_t[:], pw[:])
                nc.sync.dma_start(out=xw_dram[m], in_=xw_t[:])
                nc.vector.tensor_copy(stats_sb[:, m * 8:(m + 1) * 8], ps[:])
                nc.vector.tensor_copy(stats_r[:, m * 8:(m + 1) * 8], ps[:])

        # ---- Phase A2: build SL broadcast [128, nl] from local s_l ----
        with tc.tile_pool(name="psA2", bufs=1, space="PSUM") as psA2:
            ch = _chunks(nl)
            ptrs = [psA2.tile([1, c[1]], F32, tag=f"psl{ci}",
                              name=f"psl{ci}")
                    for ci, c in enumerate(ch)]
            for t in range(IT):
                ci, off = divmod(t * 128, 512)
                # transpose stats col (s_l of local tile t) -> row chunk
                nc.tensor.matmul(ptrs[ci][0:1, off:off + 128],
                                 stats_sb[:, t * 8:t * 8 + 1],
                                 ident_sb[:], start=True, stop=True)
            for ci, (o, w) in enumerate(ch):
                nc.vector.tensor_copy(sl_row[0:1, o:o + w], ptrs[ci][0:1, :])
            for ci, (o, w) in enumerate(ch):
                pb = psA2.tile([128, w], F32, tag="pslb")
                nc.tensor.matmul(pb[:], ones_row[:], sl_row[0:1, o:o + w],
                                 start=True, stop=True)
                nc.vector.tensor_copy(slb_sb[:, o:o + w], pb[:])

        # ---- Phase B: x_b = adj_b @ xW ; gb = vb^T adj_bT ----
        # gb done row-oriented: lhsT is the tiny vb vector (cheap weight
        # load), adjacency streams as the moving operand.
        with tc.tile_pool(name="adjB", bufs=3) as adjp, \
             tc.tile_pool(name="rhsB", bufs=3) as rhsp, \
             tc.tile_pool(name="psB", bufs=1, space="PSUM") as psB:
            pb_acc = [psB.tile([128, d], F32, tag=f"pb{i}", name=f"pb{i}")
                      for i in range(IT)]
            chn = _chunks(nl)
            pgb = [psB.tile([1, c[1]], F32, tag=f"pgb{ci}", name=f"pgb{ci}")
                   for ci, c in enumerate(chn)]
            for j in range(JT):
                at = adjp.tile([128, nl], mm_dt, tag="adj")
                nc.sync.dma_start(out=at[:], in_=adjbt_dram[j])
                xw_t = rhsp.tile([128, d], mm_dt, tag="xw")
                nc.sync.dma_start(out=xw_t[:], in_=xw_dram[j])
                st, sp = (j == 0), (j == JT - 1)
                vb = stats_r[:, j * 8 + 4:j * 8 + 5]
                for i in range(IT):
                    mm(pb_acc[i][:], at[:, i * 128:(i + 1) * 128], xw_t[:], st, sp)
                for ci, (o, w) in enumerate(chn):
                    mm(pgb[ci][:], vb, at[:, o:o + w], st, sp)
            for i in range(IT):
                nc.scalar.copy(xb_sb[:, i * d:(i + 1) * d], pb_acc[i][:])
            for ci, (o, w) in enumerate(chn):
                nc.vector.tensor_copy(gb_row[0:1, o:o + w], pgb[ci][0:1, :])

        # ---- Phase C: e = adj_a * exp(-lrelu(s)); y_a = e^T.T @ h ----
        # rowsum and ga are row-oriented with zero-padded M=2 weights:
        # pass1 lhsT=[1|0] rhs=e -> row0 += rowsum; pass2 lhsT=[0|va]
        # rhs=adj -> row1 += ga. Disjoint rows of one accumulator pair.
        with tc.tile_pool(name="adjC", bufs=3) as adjp, \
             tc.tile_pool(name="ewC", bufs=3) as ewp, \
             tc.tile_pool(name="psC", bufs=1, space="PSUM") as psC:
            pc_acc = [psC.tile([128, d], F32, tag=f"pc{i}", name=f"pc{i}")
                      for i in range(IT)]
            chn = _chunks(nl)
            prg = [psC.tile([2, c[1]], F32, tag=f"prg{ci}", name=f"prg{ci}")
                   for ci, c in enumerate(chn)]
            for j in range(JT):
                at = adjp.tile([128, nl], mm_dt, tag="adj")
                nc.sync.dma_start(out=at[:], in_=adjat_dram[j])
                s_r = stats_sb[:, j * 8 + 1:j * 8 + 2]
                m_t = ewp.tile([128, nl], F32, tag="m")
                if lrelu_on_act:
                    nc.scalar.activation(m_t[:], slb_sb[:], AF.Prelu,
                                         bias=s_r, alpha=0.01)
                else:
                    nc.vector.tensor_scalar_add(m_t[:], slb_sb[:], s_r)
                    nc.vector.scalar_tensor_tensor(m_t[:], m_t[:], 0.01, m_t[:],
                                                   op0=ALU.mult, op1=ALU.max)
                # w = exp(-m), in place
                nc.scalar.activation(m_t[:], m_t[:], AF.Exp, scale=neg1[:])
                e_t = ewp.tile([128, nl], mm_dt, tag="e")
                nc.vector.tensor_tensor(e_t[:], m_t[:], at[:], op=ALU.mult)
                st, sp = (j == 0), (j == JT - 1)
                zva = stats_r[:, j * 8 + 2:j * 8 + 4]
                for i in range(IT):
                    mm(pc_acc[i][:], e_t[:, i * 128:(i + 1) * 128],
                       h_sb[:, j * d:(j + 1) * d], st, sp)
                for ci, (o, w) in enumerate(chn):
                    mm(prg[ci][:], onespad[:], e_t[:, o:o + w], st, False)
                    mm(prg[ci][:], zva, at[:, o:o + w], False, sp)
            for i in range(IT):
                nc.scalar.copy(xa_sb[:, i * d:(i + 1) * d], pc_acc[i][:])
            for ci, (o, w) in enumerate(chn):
                nc.vector.tensor_copy(rg_rows[0:2, o:o + w], prg[ci][0:2, :])

        # ---- Phase D: transpose stat rows to columns, gates, combine ----
        with tc.tile_pool(name="psD", bufs=1, space="PSUM") as psD, \
             tc.tile_pool(name="outD", bufs=2) as outp:
            pT = psD.tile([128, 3 * IT], F32, tag="pT")
            for i in range(IT):
                # transpose [rs; ga] pair: K=2 against 2x2 identity
                nc.tensor.matmul(pT[:, 2 * i:2 * i + 2],
                                 rg_rows[0:2, i * 128:(i + 1) * 128],
                                 ident_sb[0:2, 0:2], start=True, stop=True)
                nc.tensor.matmul(pT[:, 2 * IT + i:2 * IT + i + 1],
                                 gb_row[0:1, i * 128:(i + 1) * 128],
                                 ones_row[0:1, 0:1], start=True, stop=True)
            nc.vector.tensor_copy(g_sb[:], pT[:])
            for i in range(IT):
                # recip(rowsum + 1e-5)
                nc.vector.tensor_scalar_add(gate_sb[:, 3 * IT + i:3 * IT + i + 1],
                                            g_sb[:, 2 * i:2 * i + 1], 1e-5)
                nc.vector.reciprocal(gate_sb[:, i:i + 1],
                                     gate_sb[:, 3 * IT + i:3 * IT + i + 1])
                # sig_a = sigmoid(ga + wa2x + ba)
                nc.vector.tensor_tensor(gate_sb[:, 3 * IT + i:3 * IT + i + 1],
                                        g_sb[:, 2 * i + 1:2 * i + 2],
                                        stats_sb[:, i * 8 + 5:i * 8 + 6],
                                        op=ALU.add)
                nc.scalar.activation(gate_sb[:, IT + i:IT + i + 1],
                                     gate_sb[:, 3 * IT + i:3 * IT + i + 1],
                                     AF.Sigmoid, bias=ba_sb[:])
                # sig_b = sigmoid(gb + wb2x + bb)
                nc.vector.tensor_tensor(gate_sb[:, 3 * IT + i:3 * IT + i + 1],
                                        g_sb[:, 2 * IT + i:2 * IT + i + 1],
                                        stats_sb[:, i * 8 + 6:i * 8 + 7],
                                        op=ALU.add)
                nc.scalar.activation(gate_sb[:, 2 * IT + i:2 * IT + i + 1],
                                     gate_sb[:, 3 * IT + i:3 * IT + i + 1],
                                     AF.Sigmoid, bias=bb_sb[:])
            for i in range(IT):
                u_t = outp.tile([128, d], F32, tag="u")
                # u = sig_a * (x_a_raw * recip)
                nc.vector.tensor_scalar(u_t[:], xa_sb[:, i * d:(i + 1) * d],
                                        gate_sb[:, i:i + 1],
                                        gate_sb[:, IT + i:IT + i + 1],
                                        op0=ALU.mult, op1=ALU.mult)
                t_t = outp.tile([128, d], F32, tag="t")
                # t = x_b_raw + b_gcnb
                nc.vector.tensor_tensor(t_t[:], xb_sb[:, i * d:(i + 1) * d],
                                        bbias_sb[:], op=ALU.add)
                # y = sigmoid(t * sig_b + u)
                nc.vector.scalar_tensor_tensor(t_t[:], t_t[:],
                                               gate_sb[:, 2 * IT + i:2 * IT + i + 1],
                                               u_t[:], op0=ALU.mult, op1=ALU.add)
                y_t = outp.tile([128, d], F32, tag="y")
                nc.scalar.activation(y_t[:], t_t[:], AF.Sigmoid)
                nc.sync.dma_start(out=out_dram[i * 128:(i + 1) * 128, :],
                                  in_=y_t[:])

    nc.compile()
    return nc


def make_r_matrix(W_sa, a_sa, W_gcnb, Wa, Wb, d):
    cols = np.zeros((d, 8), dtype=np.float32)
    cols[:, 0] = W_sa @ a_sa[0, :d]
    cols[:, 1] = W_sa @ a_sa[0, d:]
    # col 2 stays zero (zero-pad for the [0|va] gate weight pair)
    cols[:, 3] = Wa[0, :d]
    cols[:, 4] = Wb[0, :d]
    cols[:, 5] = Wa[0, d:]
    cols[:, 6] = Wb[0, d:]
    return np.ascontiguousarray(
        np.concatenate([W_sa, W_gcnb, cols], axis=1)).astype(np.float32)


def make_core_inputs(x, adj_a, adj_b, R, b_gcnb, n, d, nl, core):
    JT, KT = n // 128, d // 128
    rows = np.arange(core * nl, (core + 1) * nl)
    perm = np.concatenate([rows, np.arange(0, core * nl),
                           np.arange((core + 1) * nl, n)])
    xp = x[perm]
    xt = np.ascontiguousarray(
        xp.reshape(JT, 128, KT, 128).transpose(0, 2, 3, 1))
    adjat = np.ascontiguousarray(adj_a[rows][:, perm].T).reshape(JT, 128, nl)
    adjbt = np.ascontiguousarray(adj_b[rows][:, perm].T).reshape(JT, 128, nl)
    return {
        "xt": xt.astype(np.float32),
        "rmat": R.reshape(KT, 128, 2 * d + 8),
        "adjat": adjat.astype(np.float32),
        "adjbt": adjbt.astype(np.float32),
        "bbias": np.ascontiguousarray(
            np.broadcast_to(b_gcnb, (128, d))).astype(np.float32),
        "ident": np.eye(128, dtype=np.float32),
    }


_CACHE = {}


def _install_ntff_hook():
    """Dev-only: register the axon NTFF profile hook so trace=True works.

    The agent image's antenv package lacks axon_hooks; synthesize it and
    wire trn_boot's ctypes-based hook to /opt/axon/libaxon_pjrt.so.
    """
    import sys
    import types
    try:
        from antenv import axon_hooks  # noqa: F401
        return
    except ImportError:
        pass
    import antenv
    mod = types.ModuleType("antenv.axon_hooks")
    _h = [None]
    mod.get_axon_ntff_profile_hook = lambda: _h[0]
    mod.set_axon_ntff_profile_hook = lambda hook: _h.__setitem__(0, hook)
    sys.modules["antenv.axon_hooks"] = mod
    antenv.axon_hooks = mod
    from trn_agent_boot.trn_boot import _ntff_profile_via_ctypes
    mod.set_axon_ntff_profile_hook(
        _ntff_profile_via_ctypes("/opt/axon/libaxon_pjrt.so"))


def kernel(x, adj_a, adj_b, W_sa, a_sa, W_gcnb, b_gcnb, Wa, ba, Wb, bb,
           _trace=False, _trace_kwargs=None):
    from concourse.bass_utils import run_bass_kernel_spmd
    if _trace:
        _install_ntff_hook()

    n, d = x.shape
    nl = n // N_CORES
    R = make_r_matrix(W_sa, a_sa, W_gcnb, Wa, Wb, d)

    key = (n, d, nl, float(ba[0]), float(bb[0]))
    if key not in _CACHE:
        _CACHE[key] = build_program(n, d, nl, float(ba[0]), float(bb[0]))
    nc = _CACHE[key]

    in_maps = [make_core_inputs(x, adj_a, adj_b, R, b_gcnb, n, d, nl, c)
               for c in range(N_CORES)]
    res = run_bass_kernel_spmd(nc, in_maps, list(range(N_CORES)),
                               trace=_trace, **(_trace_kwargs or {}))
    out = np.empty((n, d), dtype=np.float32)
    for c in range(N_CORES):
        out[c * nl:(c + 1) * nl] = res.results[c]["out"]
    if _trace:
        kernel._last_results = res
    return out


# revision 17
# speedup vs baseline: 1.4049x; 1.0685x over previous
"""HGCN layer kernel for Trainium2, 8 NeuronCores, row-sharded SPMD.

Reference computation (N=6144, D=512):
    type_sum_a = adj_a @ x ; type_sum_b = adj_b @ x
    attn_a = sigmoid(cat[ts_a, x] @ Wa.T + ba) ; attn_b likewise
    h = x @ W_sa ; s_l = h @ a_sa[:512] ; s_r = h @ a_sa[512:]
    scores[i,j] = s_l[i] + s_r[j]
    e = adj_a * exp(-leaky_relu(scores, 0.01)) ; attn = e / (rowsum(e)+1e-5)
    x_a = attn @ h ; x_b = adj_b @ (x @ W_gcnb) + b_gcnb
    out = sigmoid(attn_a * x_a + attn_b * x_b)

Kernel strategy (per core, NL=768 local rows):
  - R = [W_sa | W_gcnb | W_sa@a_l | W_sa@a_r | Wa1.T | Wb1.T | Wa2.T | Wb2.T]
    HX = x @ R computed replicated on every core (needs full h/xW anyway).
    Gates reassociate (adj@x)@W1.T -> adj@(x@W1.T) so the NxN gate matmuls
    shrink to N-vector contractions folded into PE side-passes.
  - e computed in transposed layout [j(part), i(free)] so it is directly the
    lhsT of the attention matmul; adjacency is passed in pre-transposed,
    per-core-permuted (local rows first) so one SPMD program serves all cores.
  - rowsum(e) via ones-vector lhsT pass; division applied after the matmul.
  - float32r matmuls (4x fp32 PE rate at N>=256).
"""

import numpy as np
from contextlib import ExitStack

import concourse.bass as bass
import concourse.bacc as bacc
import concourse.mybir as mybir
import concourse.tile as tile

F32 = mybir.dt.float32
F32R = mybir.dt.float32r
BF16 = mybir.dt.bfloat16
AF = mybir.ActivationFunctionType
ALU = mybir.AluOpType

N_CORES = 8


def _chunks(total, size=512):
    out = []
    o = 0
    while o < total:
        out.append((o, min(size, total - o)))
        o += size
    return out


def build_program(n, d, nl, ba, bb, dt_a=F32R, dt_bc=BF16,
                  lrelu_on_act=False):
    """Build the SPMD Bass program. Returns nc.

    n: total nodes, d: feature dim, nl: local rows per core.
    ba/bb: python-float gate biases (baked in).
    """
    JT = n // 128   # j tiles (contraction/node axis)
    IT = nl // 128  # local row tiles
    KT = d // 128   # feature k tiles
    NR = 2 * d + 8  # columns of R
    # stats cols: 0=s_l 1=s_r 2=zero 3=va 4=vb 5=wa2x 6=wb2x 7=pad

    nc = bacc.Bacc("TRN2", target_bir_lowering=False, debug=False,
                   num_devices=N_CORES)

    xt_dram = nc.dram_tensor("xt", [JT, KT, 128, 128], dt_a, kind="ExternalInput")
    r_dram = nc.dram_tensor("rmat", [KT, 128, NR], dt_a, kind="ExternalInput")
    adjat_dram = nc.dram_tensor("adjat", [JT, 128, nl], dt_bc, kind="ExternalInput")
    adjbt_dram = nc.dram_tensor("adjbt", [JT, 128, nl], dt_bc, kind="ExternalInput")
    bbias_dram = nc.dram_tensor("bbias", [128, d], F32, kind="ExternalInput")
    ident_dram = nc.dram_tensor("ident", [128, 128], F32, kind="ExternalInput")
    out_dram = nc.dram_tensor("out", [nl, d], F32, kind="ExternalOutput")

    xw_dram = nc.dram_tensor("xw_scratch", [JT, 128, d], dt_bc)

    def mm(out, lhsT, rhs, start, stop, skip_group_check=False):
        nc.tensor.matmul(out, lhsT, rhs, start=start, stop=stop,
                         skip_group_check=skip_group_check)

    with tile.TileContext(nc) as tc, ExitStack() as ctx:
        const = ctx.enter_context(tc.tile_pool(name="const", bufs=1))

        r_sb = const.tile([128, KT, NR], dt_a, tag="r")
        h_sb = const.tile([128, JT * d], dt_bc, tag="h")
        stats_sb = const.tile([128, JT * 8], F32, tag="stats")
        stats_r = const.tile([128, JT * 8], dt_bc, tag="stats_r")
        slb_sb = const.tile([128, nl], F32, tag="slb")
        xb_sb = const.tile([128, IT * d], F32, tag="xb")
        xa_sb = const.tile([128, IT * d], F32, tag="xa")
        bbias_sb = const.tile([128, d], F32, tag="bbias")
        ident_sb = const.tile([128, 128], F32, tag="ident")
        onespad = const.tile([128, 2], dt_bc, tag="onespad")
        onespad_f = const.tile([128, 2], F32, tag="onespadf")
        ones_row = const.tile([1, 128], F32, tag="ones_r")
        neg1 = const.tile([128, 1], F32, tag="neg1")
        ba_sb = const.tile([128, 1], F32, tag="ba")
        bb_sb = const.tile([128, 1], F32, tag="bb")
        sl_row = const.tile([1, nl], F32, tag="sl_row")
        g_sb = const.tile([128, 3 * IT], F32, tag="g")  # rs|ga|gb cols
        rg_rows = const.tile([2, nl], F32, tag="rg_rows")  # row0=rs row1=ga
        gb_row = const.tile([1, nl], F32, tag="gb_row")
        gate_sb = const.tile([128, 4 * IT], F32, tag="gate")
        # gate_sb cols: [0:IT]=recip(rowsum), [IT:2IT]=sig_a, [2IT:3IT]=sig_b,
        # [3IT:4IT]=scratch

        for k in range(KT):
            nc.sync.dma_start(out=r_sb[:, k, :], in_=r_dram[k])
        nc.sync.dma_start(out=bbias_sb[:], in_=bbias_dram[:])
        nc.sync.dma_start(out=ident_sb[:], in_=ident_dram[:])
        nc.vector.memset(onespad_f[:], 0.0)
        nc.vector.memset(onespad_f[:, 0:1], 1.0)
        nc.vector.tensor_copy(onespad[:], onespad_f[:])
        nc.vector.memset(ones_row[:], 1.0)
        nc.vector.memset(neg1[:], -1.0)
        nc.vector.memset(ba_sb[:], float(ba))
        nc.vector.memset(bb_sb[:], float(bb))

        # ---- Phase A: HX = x @ R (replicated over all n nodes) ----
        with tc.tile_pool(name="xt_pool", bufs=3) as xtp, \
             tc.tile_pool(name="xw_out", bufs=3) as xwop, \
             tc.tile_pool(name="psA", bufs=2, space="PSUM") as psA:
            for m in range(JT):
                xt_t = xtp.tile([128, KT * 128], dt_a, tag="xt")
                for k in range(KT):
                    nc.sync.dma_start(out=xt_t[:, k * 128:(k + 1) * 128],
                                      in_=xt_dram[m, k])
                ph = psA.tile([128, d], F32, tag="ph")
                pw = psA.tile([128, d], F32, tag="pw")
                ps = psA.tile([128, 8], F32, tag="ps")
                for k in range(KT):
                    lhsT = xt_t[:, k * 128:(k + 1) * 128]
                    st, sp = (k == 0), (k == KT - 1)
                    mm(ph[:], lhsT, r_sb[:, k, 0:d], st, sp)
                    mm(pw[:], lhsT, r_sb[:, k, d:2 * d], st, sp)
                    mm(ps[:], lhsT, r_sb[:, k, 2 * d:NR], st, sp)
                nc.scalar.copy(h_sb[:, m * d:(m + 1) * d], ph[:])
                xw_t = xwop.tile([128, d], dt_bc, tag="xwo")
                nc.scalar.copy(xw_t[:], pw[:])
                nc.sync.dma_start(out=xw_dram[m], in_=xw_t[:])
                nc.vector.tensor_copy(stats_sb[:, m * 8:(m + 1) * 8], ps[:])
                nc.vector.tensor_copy(stats_r[:, m * 8:(m + 1) * 8], ps[:])

        # ---- Phase A2: build SL broadcast [128, nl] from local s_l ----
        with tc.tile_pool(name="psA2", bufs=1, space="PSUM") as psA2:
            ch = _chunks(nl)
            ptrs = [psA2.tile([1, c[1]], F32, tag=f"psl{ci}",
                              name=f"psl{ci}")
                    for ci, c in enumerate(ch)]
            for t in range(IT):
                ci, off = divmod(t * 128, 512)
                # transpose stats col (s_l of local tile t) -> row chunk
                nc.tensor.matmul(ptrs[ci][0:1, off:off + 128],
                                 stats_sb[:, t * 8:t * 8 + 1],
                                 ident_sb[:], start=True, stop=True)
            for ci, (o, w) in enumerate(ch):
                nc.vector.tensor_copy(sl_row[0:1, o:o + w], ptrs[ci][0:1, :])
            for ci, (o, w) in enumerate(ch):
                pb = psA2.tile([128, w], F32, tag="pslb")
                nc.tensor.matmul(pb[:], ones_row[:], sl_row[0:1, o:o + w],
                                 start=True, stop=True)
                nc.vector.tensor_copy(slb_sb[:, o:o + w], pb[:])

        # ---- Phase B: x_b = adj_b @ xW ; gb = vb^T adj_bT ----
        # gb done row-oriented: lhsT is the tiny vb vector (cheap weight
        # load), adjacency streams as the moving operand.
        with tc.tile_pool(name="adjB", bufs=3) as adjp, \
             tc.tile_pool(name="rhsB", bufs=3) as rhsp, \
             tc.tile_pool(name="psB", bufs=1, space="PSUM") as psB:
            pb_acc = [psB.tile([128, d], F32, tag=f"pb{i}", name=f"pb{i}")
                      for i in range(IT)]
            chn = _chunks(nl)
            pgb = [psB.tile([1, c[1]], F32, tag=f"pgb{ci}", name=f"pgb{ci}")
                   for ci, c in enumerate(chn)]
            for j in range(JT):
                at = adjp.tile([128, nl], dt_bc, tag="adj")
                nc.sync.dma_start(out=at[:], in_=adjbt_dram[j])
                xw_t = rhsp.tile([128, d], dt_bc, tag="xw")
                nc.sync.dma_start(out=xw_t[:], in_=xw_dram[j])
                st, sp = (j == 0), (j == JT - 1)
                vb = stats_r[:, j * 8 + 4:j * 8 + 5]
                for i in range(IT):
                    mm(pb_acc[i][:], at[:, i * 128:(i + 1) * 128], xw_t[:], st, sp)
                for ci, (o, w) in enumerate(chn):
                    mm(pgb[ci][:], vb, at[:, o:o + w], st, sp)
            for i in range(IT):
                nc.scalar.copy(xb_sb[:, i * d:(i + 1) * d], pb_acc[i][:])
            for ci, (o, w) in enumerate(chn):
                nc.vector.tensor_copy(gb_row[0:1, o:o + w], pgb[ci][0:1, :])

        # ---- Phase C: e = adj_a * exp(-lrelu(s)); y_a = e^T.T @ h ----
        # rowsum and ga are row-oriented with zero-padded M=2 weights:
        # pass1 lhsT=[1|0] rhs=e -> row0 += rowsum; pass2 lhsT=[0|va]
        # rhs=adj -> row1 += ga. Disjoint rows of one accumulator pair.
        with tc.tile_pool(name="adjC", bufs=3) as adjp, \
             tc.tile_pool(name="ewC", bufs=3) as ewp, \
             tc.tile_pool(name="psC", bufs=1, space="PSUM") as psC:
            pc_acc = [psC.tile([128, d], F32, tag=f"pc{i}", name=f"pc{i}")
                      for i in range(IT)]
            chn = _chunks(nl)
            prg = [psC.tile([2, c[1]], F32, tag=f"prg{ci}", name=f"prg{ci}")
                   for ci, c in enumerate(chn)]
            for j in range(JT):
                at = adjp.tile([128, nl], dt_bc, tag="adj")
                nc.sync.dma_start(out=at[:], in_=adjat_dram[j])
                s_r = stats_sb[:, j * 8 + 1:j * 8 + 2]
                m_t = ewp.tile([128, nl], F32, tag="m")
                if lrelu_on_act:
                    nc.scalar.activation(m_t[:], slb_sb[:], AF.Prelu,
                                         bias=s_r, alpha=0.01)
                else:
                    nc.vector.tensor_scalar_add(m_t[:], slb_sb[:], s_r)
                    nc.vector.scalar_tensor_tensor(m_t[:], m_t[:], 0.01, m_t[:],
                                                   op0=ALU.mult, op1=ALU.max)
                # w = exp(-m), in place
                nc.scalar.activation(m_t[:], m_t[:], AF.Exp, scale=neg1[:])
                e_t = ewp.tile([128, nl], dt_bc, tag="e")
                nc.vector.tensor_tensor(e_t[:], m_t[:], at[:], op=ALU.mult)
                st, sp = (j == 0), (j == JT - 1)
                zva = stats_r[:, j * 8 + 2:j * 8 + 4]
                for i in range(IT):
                    mm(pc_acc[i][:], e_t[:, i * 128:(i + 1) * 128],
                       h_sb[:, j * d:(j + 1) * d], st, sp)
                for ci, (o, w) in enumerate(chn):
                    mm(prg[ci][:], onespad[:], e_t[:, o:o + w], st, False)
                    mm(prg[ci][:], zva, at[:, o:o + w], False, sp)
            for i in range(IT):
                nc.scalar.copy(xa_sb[:, i * d:(i + 1) * d], pc_acc[i][:])
            for ci, (o, w) in enumerate(chn):
                nc.vector.tensor_copy(rg_rows[0:2, o:o + w], prg[ci][0:2, :])

        # ---- Phase D: transpose stat rows to columns, gates, combine ----
        with tc.tile_pool(name="psD", bufs=1, space="PSUM") as psD, \
             tc.tile_pool(name="outD", bufs=2) as outp:
            pT = psD.tile([128, 3 * IT], F32, tag="pT")
            for i in range(IT):
                # transpose [rs; ga] pair: K=2 against 2x2 identity
                nc.tensor.matmul(pT[:, 2 * i:2 * i + 2],
                                 rg_rows[0:2, i * 128:(i + 1) * 128],
                                 ident_sb[0:2, 0:2], start=True, stop=True)
                nc.tensor.matmul(pT[:, 2 * IT + i:2 * IT + i + 1],
                                 gb_row[0:1, i * 128:(i + 1) * 128],
                                 ones_row[0:1, 0:1], start=True, stop=True)
            nc.vector.tensor_copy(g_sb[:], pT[:])
            for i in range(IT):
                # recip(rowsum + 1e-5)
                nc.vector.tensor_scalar_add(gate_sb[:, 3 * IT + i:3 * IT + i + 1],
                                            g_sb[:, 2 * i:2 * i + 1], 1e-5)
                nc.vector.reciprocal(gate_sb[:, i:i + 1],
                                     gate_sb[:, 3 * IT + i:3 * IT + i + 1])
                # sig_a = sigmoid(ga + wa2x + ba)
                nc.vector.tensor_tensor(gate_sb[:, 3 * IT + i:3 * IT + i + 1],
                                        g_sb[:, 2 * i + 1:2 * i + 2],
                                        stats_sb[:, i * 8 + 5:i * 8 + 6],
                                        op=ALU.add)
                nc.scalar.activation(gate_sb[:, IT + i:IT + i + 1],
                                     gate_sb[:, 3 * IT + i:3 * IT + i + 1],
                                     AF.Sigmoid, bias=ba_sb[:])
                # sig_b = sigmoid(gb + wb2x + bb)
                nc.vector.tensor_tensor(gate_sb[:, 3 * IT + i:3 * IT + i + 1],
                                        g_sb[:, 2 * IT + i:2 * IT + i + 1],
                                        stats_sb[:, i * 8 + 6:i * 8 + 7],
                                        op=ALU.add)
                nc.scalar.activation(gate_sb[:, 2 * IT + i:2 * IT + i + 1],
                                     gate_sb[:, 3 * IT + i:3 * IT + i + 1],
                                     AF.Sigmoid, bias=bb_sb[:])
            for i in range(IT):
                u_t = outp.tile([128, d], F32, tag="u")
                # u = sig_a * (x_a_raw * recip)
                nc.vector.tensor_scalar(u_t[:], xa_sb[:, i * d:(i + 1) * d],
                                        gate_sb[:, i:i + 1],
                                        gate_sb[:, IT + i:IT + i + 1],
                                        op0=ALU.mult, op1=ALU.mult)
                t_t = outp.tile([128, d], F32, tag="t")
                # t = x_b_raw + b_gcnb
                nc.vector.tensor_tensor(t_t[:], xb_sb[:, i * d:(i + 1) * d],
                                        bbias_sb[:], op=ALU.add)
                # y = sigmoid(t * sig_b + u)
                nc.vector.scalar_tensor_tensor(t_t[:], t_t[:],
                                               gate_sb[:, 2 * IT + i:2 * IT + i + 1],
                                               u_t[:], op0=ALU.mult, op1=ALU.add)
                y_t = outp.tile([128, d], F32, tag="y")
                nc.scalar.activation(y_t[:], t_t[:], AF.Sigmoid)
                nc.sync.dma_start(out=out_dram[i * 128:(i + 1) * 128, :],
                                  in_=y_t[:])

    nc.compile()
    return nc


def make_r_matrix(W_sa, a_sa, W_gcnb, Wa, Wb, d):
    cols = np.zeros((d, 8), dtype=np.float32)
    cols[:, 0] = W_sa @ a_sa[0, :d]
    cols[:, 1] = W_sa @ a_sa[0, d:]
    # col 2 stays zero (zero-pad for the [0|va] gate weight pair)
    cols[:, 3] = Wa[0, :d]
    cols[:, 4] = Wb[0, :d]
    cols[:, 5] = Wa[0, d:]
    cols[:, 6] = Wb[0, d:]
    return np.ascontiguousarray(
        np.concatenate([W_sa, W_gcnb, cols], axis=1)).astype(np.float32)


def make_core_inputs(x, adj_a, adj_b, R, b_gcnb, n, d, nl, core,
                     np_a=np.float32, np_bc=None):
    if np_bc is None:
        import ml_dtypes
        np_bc = ml_dtypes.bfloat16
    JT, KT = n // 128, d // 128
    rows = np.arange(core * nl, (core + 1) * nl)
    perm = np.concatenate([rows, np.arange(0, core * nl),
                           np.arange((core + 1) * nl, n)])
    xp = x[perm]
    xt = np.ascontiguousarray(
        xp.reshape(JT, 128, KT, 128).transpose(0, 2, 3, 1))
    adjat = np.ascontiguousarray(adj_a[rows][:, perm].T).reshape(JT, 128, nl)
    adjbt = np.ascontiguousarray(adj_b[rows][:, perm].T).reshape(JT, 128, nl)
    return {
        "xt": xt.astype(np_a),
        "rmat": R.reshape(KT, 128, 2 * d + 8).astype(np_a),
        "adjat": adjat.astype(np_bc),
        "adjbt": adjbt.astype(np_bc),
        "bbias": np.ascontiguousarray(
            np.broadcast_to(b_gcnb, (128, d))).astype(np.float32),
        "ident": np.eye(128, dtype=np.float32),
    }


_CACHE = {}


def _install_ntff_hook():
    """Dev-only: register the axon NTFF profile hook so trace=True works.

    The agent image's antenv package lacks axon_hooks; synthesize it and
    wire trn_boot's ctypes-based hook to /opt/axon/libaxon_pjrt.so.
    """
    import sys
    import types
    try:
        from antenv import axon_hooks  # noqa: F401
        return
    except ImportError:
        pass
    import antenv
    mod = types.ModuleType("antenv.axon_hooks")
    _h = [None]
    mod.get_axon_ntff_profile_hook = lambda: _h[0]
    mod.set_axon_ntff_profile_hook = lambda hook: _h.__setitem__(0, hook)
    sys.modules["antenv.axon_hooks"] = mod
    antenv.axon_hooks = mod
    from trn_agent_boot.trn_boot import _ntff_profile_via_ctypes
    mod.set_axon_ntff_profile_hook(
        _ntff_profile_via_ctypes("/opt/axon/libaxon_pjrt.so"))


def kernel(x, adj_a, adj_b, W_sa, a_sa, W_gcnb, b_gcnb, Wa, ba, Wb, bb,
           _trace=False, _trace_kwargs=None):
    from concourse.bass_utils import run_bass_kernel_spmd
    if _trace:
        _install_ntff_hook()

    n, d = x.shape
    nl = n // N_CORES
    R = make_r_matrix(W_sa, a_sa, W_gcnb, Wa, Wb, d)

    key = (n, d, nl, float(ba[0]), float(bb[0]))
    if key not in _CACHE:
        _CACHE[key] = build_program(n, d, nl, float(ba[0]), float(bb[0]))
    nc = _CACHE[key]

    in_maps = [make_core_inputs(x, adj_a, adj_b, R, b_gcnb, n, d, nl, c)
               for c in range(N_CORES)]
    res = run_bass_kernel_spmd(nc, in_maps, list(range(N_CORES)),
                               trace=_trace, **(_trace_kwargs or {}))
    out = np.empty((n, d), dtype=np.float32)
    for c in range(N_CORES):
        out[c * nl:(c + 1) * nl] = res.results[c]["out"]
    if _trace:
        kernel._last_results = res
    return out
